# revision 12
# baseline (speedup 1.0000x reference)
import numpy as np
import ml_dtypes

B, S, I, H, C = 64, 512, 256, 512, 10
NCORES = 8
BL = B // NCORES
G = 16          # steps per psum group (one full bank: 16*4*8 = 512 f32)
NG = S // G     # 32 groups per layer
D = 48          # layer-1 lag in steps
CH = 32         # xT DMA chunk (steps)

_cache = {}


def _build_nc():
    import concourse.bass as bass
    import concourse.bacc as bacc
    import concourse.tile as tile
    from concourse.bass import mybir

    f32 = mybir.dt.float32
    bf16 = mybir.dt.bfloat16
    Tanh = mybir.ActivationFunctionType.Tanh

    nc = bacc.Bacc("TRN2", target_bir_lowering=False, debug=False, num_devices=NCORES)

    xT_d = nc.dram_tensor("xT", [128, 2, S * BL], bf16, kind="ExternalInput")
    wih0_d = nc.dram_tensor("wih0", [128, 2, 4, 128], bf16, kind="ExternalInput")
    whh0_d = nc.dram_tensor("whh0", [128, 4, 4, 128], bf16, kind="ExternalInput")
    wih1_d = nc.dram_tensor("wih1", [128, 4, 4, 128], bf16, kind="ExternalInput")
    whh1_d = nc.dram_tensor("whh1", [128, 4, 4, 128], bf16, kind="ExternalInput")
    wfc_d = nc.dram_tensor("wfc", [128, 4, C], bf16, kind="ExternalInput")
    b0T_d = nc.dram_tensor("b0T", [128, 4, 128], bf16, kind="ExternalInput")
    b1T_d = nc.dram_tensor("b1T", [128, 4, 128], bf16, kind="ExternalInput")
    ones_d = nc.dram_tensor("ones", [128, G * BL], bf16, kind="ExternalInput")
    bfc_d = nc.dram_tensor("bfc", [C, 1], f32, kind="ExternalInput")
    out_d = nc.dram_tensor("out", [C, BL], f32, kind="ExternalOutput")

    with tile.TileContext(nc) as tc:
        with tc.tile_pool(name="sb", bufs=1) as sb, tc.tile_pool(
            name="ps", bufs=1, space="PSUM"
        ) as psp:
            xT = sb.tile([128, 2, S * BL], bf16)
            out0 = sb.tile([128, S, 4, BL], bf16)
            h1 = sb.tile([128, 2, 4, BL], bf16)
            wih0 = sb.tile([128, 2, 4, 128], bf16)
            whh0 = sb.tile([128, 4, 4, 128], bf16)
            wih1 = sb.tile([128, 4, 4, 128], bf16)
            whh1 = sb.tile([128, 4, 4, 128], bf16)
            wfc = sb.tile([128, 4, C], bf16)
            b0T = sb.tile([128, 4, 128], bf16)
            b1T = sb.tile([128, 4, 128], bf16)
            ones = sb.tile([128, G * BL], bf16)
            bfc = sb.tile([C, 1], f32)
            fco = sb.tile([C, BL], f32)

            nc.sync.dma_start(wih0[:], wih0_d[:])
            nc.sync.dma_start(b0T[:], b0T_d[:])
            nc.sync.dma_start(ones[:], ones_d[:])
            nc.sync.dma_start(xT[:, :, 0 : CH * BL], xT_d[:, :, 0 : CH * BL])
            nc.sync.dma_start(whh0[:], whh0_d[:])
            nc.sync.dma_start(
                xT[:, :, CH * BL : 4 * CH * BL], xT_d[:, :, CH * BL : 4 * CH * BL]
            )
            nc.sync.dma_start(xT[:, :, 4 * CH * BL :], xT_d[:, :, 4 * CH * BL :])
            for t_sb, t_d in [
                (wih1, wih1_d), (whh1, whh1_d), (b1T, b1T_d),
                (wfc, wfc_d), (bfc, bfc_d),
            ]:
                nc.sync.dma_start(t_sb[:], t_d[:])

            # bank layout [128, jc, G*BL]: every matmul dst is contiguous
            p0 = [psp.tile([128, 4, G * BL], f32, name=f"p0_{i}") for i in range(2)]
            p1 = [psp.tile([128, 4, G * BL], f32, name=f"p1_{i}") for i in range(3)]
            fcps = psp.tile([128, BL], f32, name="fcps")

            # --- pre-GEMM jobs: fill psum group banks directly ---
            # PSUM start=True zeroes the WHOLE 2KB bank (zero region), so
            # exactly ONE start per bank cycle: the jc=0 bias job. All other
            # writes land on pending-zero elements and accumulate correctly.
            # Exactly one stop per bank cycle: last recurrence MM of slot 15.
            def pre0_job(g, jc, kc):
                bank = p0[g % 2]
                if kc == "b":
                    nc.tensor.matmul(
                        bank[:, jc, :], b0T[:, jc, :], ones[:],
                        start=(jc == 0), stop=False, skip_group_check=True,
                    )
                else:
                    nc.tensor.matmul(
                        bank[:, jc, :],
                        wih0[:, kc, jc, :],
                        xT[:, kc, g * G * BL : (g + 1) * G * BL],
                        start=False, stop=False, skip_group_check=True,
                    )

            def pre1_job(g, jc, kc):
                bank = p1[g % 3]
                if kc == "b":
                    nc.tensor.matmul(
                        bank[:, jc, :], b1T[:, jc, :], ones[:],
                        start=(jc == 0), stop=False, skip_group_check=True,
                    )
                else:
                    nc.tensor.matmul(
                        bank[:, jc, :],
                        wih1[:, kc, jc, :],
                        out0[:, g * G : (g + 1) * G, kc, :],
                        start=False, stop=False, skip_group_check=True,
                    )

            def l0_step(t):
                bank = p0[(t // G) % 2]
                sl = t % G
                if t > 0:
                    for kc in range(4):
                        for jc in range(4):
                            nc.tensor.matmul(
                                bank[:, jc, sl * BL : (sl + 1) * BL],
                                whh0[:, kc, jc, :],
                                out0[:, t - 1, kc, :],
                                start=False,
                                stop=(kc == 3 and jc == 3 and sl == G - 1),
                                skip_group_check=True,
                            )
                nc.scalar.activation(
                    out0[:, t, :, :], bank[:, :, sl * BL : (sl + 1) * BL], Tanh
                )

            def l1_step(s):
                bank = p1[(s // G) % 3]
                sl = s % G
                if s > 0:
                    for kc in range(4):
                        for jc in range(4):
                            nc.tensor.matmul(
                                bank[:, jc, sl * BL : (sl + 1) * BL],
                                whh1[:, kc, jc, :],
                                h1[:, (s - 1) % 2, kc, :],
                                start=False,
                                stop=(kc == 3 and jc == 3 and sl == G - 1),
                                skip_group_check=True,
                            )
                nc.scalar.activation(
                    h1[:, s % 2, :, :], bank[:, :, sl * BL : (sl + 1) * BL], Tanh
                )

            # --- static schedule of pre jobs into 2 slots/iteration ---
            # kc encoding: ints are real kc chunks; NB0/NB1 means bias job.
            # (release, deadline, g, jc, kc): release spreads each group's
            # jobs evenly over its window so fill work is available every
            # iteration (greedy-early popping bunches jobs and leaves stall
            # holes); release also gates bank reuse (WAR) and out0 readiness
            # -- emitting earlier would deadlock the in-order PE.
            # kc == -1 encodes the bias job (must be first per jc; the jc=0
            # bias is the bank's single start=True and must lead the group).
            q0 = []
            for g in range(1, NG):
                w0 = max(0, 16 * g - 16)
                jobs = [(jc, kc) for jc in range(4) for kc in [-1, 0, 1]]
                for j, (jc, kc) in enumerate(jobs):
                    q0.append((w0 + (j * 16) // len(jobs), 16 * g, g, jc, kc))
            q1 = []
            for g in range(NG):
                w1 = 16 * g + 16
                jobs = [(jc, kc) for jc in range(4) for kc in [-1, 0, 1, 2, 3]]
                for j, (jc, kc) in enumerate(jobs):
                    q1.append((w1 + (j * 32) // len(jobs), 16 * g + D, g, jc, kc))
            q0.sort()
            q1.sort()
            i0 = [0]
            i1 = [0]

            def pop(t):
                c0 = q0[i0[0]] if i0[0] < len(q0) else None
                c1 = q1[i1[0]] if i1[0] < len(q1) else None
                if c0 is not None and c0[0] > t:
                    c0 = None  # not yet released
                if c1 is not None and c1[0] > t:
                    c1 = None
                if c0 is None and c1 is None:
                    return
                if c1 is None or (c0 is not None and c0[1] <= c1[1]):
                    _, dl, g, jc, kc = c0
                    assert dl > t, f"pre0 job past deadline: {c0} at t={t}"
                    i0[0] += 1
                    pre0_job(g, jc, "b" if kc == -1 else kc)
                else:
                    _, dl, g, jc, kc = c1
                    assert dl > t, f"pre1 job past deadline: {c1} at t={t}"
                    i1[0] += 1
                    pre1_job(g, jc, "b" if kc == -1 else kc)

            # HAM warmup: ~8us of dense independent matmuls so the PE enters
            # the main loop at 2.4GHz (K=8/8). Cold drains lengthen the tanh
            # hop past the fill window and the loop then self-sustains cold.
            for i in range(80):
                b = i // 20          # p1 bank, idle until t>=16
                j = (i % 20) % 4
                nc.tensor.matmul(
                    p1[b % 3][:, j, :] if b < 3 else p0[1][:, j, :],
                    wih0[:, 0, j, :],
                    xT[:, 0, 0 : G * BL],
                    start=(i % 20 == 0),
                    stop=(i % 20 == 19),
                    skip_group_check=True,
                )

            # prologue: fill l0 group 0 entirely (bias first: it is the start)
            for jc in range(4):
                for kc in ["b", 0, 1]:
                    pre0_job(0, jc, kc)

            for t in range(S + D):
                if t < S:
                    l0_step(t)
                pop(t)
                if t >= D:
                    l1_step(t - D)
                pop(t)

            assert i0[0] == len(q0) and i1[0] == len(q1), (
                f"unscheduled pre jobs: {len(q0) - i0[0]} pre0, {len(q1) - i1[0]} pre1"
            )

            # --- FC head on final h1 (step 511 -> slot 1) ---
            for kc in range(4):
                nc.tensor.matmul(
                    fcps[0:C, :], wfc[:, kc, :], h1[:, 1, kc, :],
                    start=(kc == 0), stop=(kc == 3),
                )
            nc.vector.tensor_scalar_add(fco[:], fcps[0:C, :], bfc[:])
            nc.sync.dma_start(out_d[:], fco[:])

    nc.compile()
    return nc


def _prep_inputs(inputs):
    bf = ml_dtypes.bfloat16
    w_ih0 = inputs["w_ih0"]
    w_hh0 = inputs["w_hh0"]
    w_ih1 = inputs["w_ih1"]
    w_hh1 = inputs["w_hh1"]
    w_fc = inputs["w_fc"]

    def lhsT_4(w, n_kc):
        # w: [512, n_kc*128] -> [kp, kc, jc, jp]
        return np.ascontiguousarray(
            w.reshape(4, 128, n_kc, 128).transpose(3, 2, 0, 1)
        ).astype(bf)

    def biasT(b):
        # [kp, jc, jp], only kp=0 row nonzero
        out = np.zeros((128, 4, 128), np.float32)
        out[0] = b.reshape(4, 128)
        return out.astype(bf)

    ones = np.zeros((128, G * BL), np.float32)
    ones[0] = 1.0

    shared = {
        "wih0": lhsT_4(w_ih0, 2),
        "whh0": lhsT_4(w_hh0, 4),
        "wih1": lhsT_4(w_ih1, 4),
        "whh1": lhsT_4(w_hh1, 4),
        "wfc": np.ascontiguousarray(
            w_fc.reshape(C, 4, 128).transpose(2, 1, 0)
        ).astype(bf),
        "b0T": biasT(inputs["b_ih0"] + inputs["b_hh0"]),
        "b1T": biasT(inputs["b_ih1"] + inputs["b_hh1"]),
        "ones": ones.astype(bf),
        "bfc": inputs["b_fc"].reshape(C, 1).astype(np.float32),
    }
    x = inputs["x"]
    in_maps = []
    for c in range(NCORES):
        xs = x[c * BL : (c + 1) * BL]  # [b, t, i]
        xT = (
            np.ascontiguousarray(
                xs.transpose(2, 1, 0).reshape(2, 128, S * BL).transpose(1, 0, 2)
            )
        ).astype(bf)
        m = dict(shared)
        m["xT"] = xT
        in_maps.append(m)
    return in_maps


def kernel(**inputs):
    from concourse import bass_utils

    if "nc" not in _cache:
        _cache["nc"] = _build_nc()
    nc = _cache["nc"]
    in_maps = _prep_inputs(inputs)
    res = bass_utils.run_bass_kernel_spmd(nc, in_maps, core_ids=list(range(NCORES)))
    y = np.concatenate(
        [np.asarray(res.results[c]["out"]).T for c in range(NCORES)], axis=0
    )
    return y.astype(np.float32)


# revision 16
# speedup vs baseline: 1.0040x; 1.0040x over previous
import numpy as np
import ml_dtypes

B, S, I, H, C = 64, 512, 256, 512, 10
NCORES = 8
BL = B // NCORES
CH = 32
D = 48
NCH = S // CH

_cache = {}


def _build_nc():
    from collections import deque

    import concourse.bass as bass
    import concourse.bacc as bacc
    import concourse.tile as tile
    from concourse.bass import mybir

    f32 = mybir.dt.float32
    bf16 = mybir.dt.bfloat16
    Tanh = mybir.ActivationFunctionType.Tanh

    nc = bacc.Bacc("TRN2", target_bir_lowering=False, debug=False, num_devices=NCORES)

    xT_d = nc.dram_tensor("xT", [128, 2, S * BL], bf16, kind="ExternalInput")
    wih0_d = nc.dram_tensor("wih0", [128, 2, 4, 128], bf16, kind="ExternalInput")
    whh0_d = nc.dram_tensor("whh0", [128, 4, 4, 128], bf16, kind="ExternalInput")
    wih1_d = nc.dram_tensor("wih1", [128, 4, 4, 128], bf16, kind="ExternalInput")
    whh1_d = nc.dram_tensor("whh1", [128, 4, 4, 128], bf16, kind="ExternalInput")
    wfc_d = nc.dram_tensor("wfc", [128, 4, C], bf16, kind="ExternalInput")
    b0_d = nc.dram_tensor("b0", [128, 4], f32, kind="ExternalInput")
    b1_d = nc.dram_tensor("b1", [128, 4], f32, kind="ExternalInput")
    bfc_d = nc.dram_tensor("bfc", [C, 1], f32, kind="ExternalInput")
    id_d = nc.dram_tensor("ident", [128, 128], bf16, kind="ExternalInput")
    out_d = nc.dram_tensor("out", [C, BL], f32, kind="ExternalOutput")

    with tile.TileContext(nc) as tc:
        with tc.tile_pool(name="sb", bufs=1) as sb, tc.tile_pool(
            name="ps", bufs=1, space="PSUM"
        ) as psp:
            xT = sb.tile([128, 2, S * BL], bf16)
            pre0 = sb.tile([128, S, 4, BL], bf16)
            out0 = sb.tile([128, S, 4, BL], bf16)
            pre1 = sb.tile([128, S, 4, BL], bf16)
            wih0 = sb.tile([128, 2, 4, 128], bf16)
            whh0 = sb.tile([128, 4, 4, 128], bf16)
            wih1 = sb.tile([128, 4, 4, 128], bf16)
            whh1 = sb.tile([128, 4, 4, 128], bf16)
            wfc = sb.tile([128, 4, C], bf16)
            b0 = sb.tile([128, 4], f32)
            b1 = sb.tile([128, 4], f32)
            bfc = sb.tile([C, 1], f32)
            ident = sb.tile([128, 128], bf16)
            h1 = sb.tile([128, 2, 4, BL], bf16)
            fco = sb.tile([C, BL], f32)

            nc.sync.dma_start(wih0[:], wih0_d[:])
            nc.sync.dma_start(
                xT[:, :, 0 : CH * BL], xT_d[:, :, 0 : CH * BL]
            )
            nc.sync.dma_start(b0[:], b0_d[:])
            nc.sync.dma_start(ident[:], id_d[:])
            nc.sync.dma_start(
                xT[:, :, CH * BL : 4 * CH * BL], xT_d[:, :, CH * BL : 4 * CH * BL]
            )
            nc.sync.dma_start(whh0[:], whh0_d[:])
            nc.sync.dma_start(
                xT[:, :, 4 * CH * BL :], xT_d[:, :, 4 * CH * BL :]
            )
            for t_sb, t_d in [
                (wih1, wih1_d), (whh1, whh1_d), (b1, b1_d),
                (wfc, wfc_d), (bfc, bfc_d),
            ]:
                nc.sync.dma_start(t_sb[:], t_d[:])

            gps = [psp.tile([128, 64, BL], f32, name=f"gps{i}") for i in range(4)]
            sps = [psp.tile([128, 4, 4, BL], f32, name=f"sps{i}") for i in range(4)]

            def g0_mm(k, jc, kc):
                t0 = k * CH
                ps = gps[jc]
                nc.tensor.matmul(
                    ps[:, 0:CH, :],
                    wih0[:, kc, jc, :],
                    xT[:, kc, t0 * BL : (t0 + CH) * BL],
                    start=(kc == 0),
                    stop=(kc == 1),
                )
                if kc == 1:
                    nc.vector.tensor_scalar_add(
                        pre0[:, t0 : t0 + CH, jc, :], ps[:, 0:CH, :],
                        b0[:, jc : jc + 1],
                    )

            def g1_mm(k, jc, kc):
                t0 = k * CH
                ps = gps[jc]
                nc.tensor.matmul(
                    ps[:, 0:CH, :],
                    wih1[:, kc, jc, :],
                    out0[:, t0 : t0 + CH, kc, :],
                    start=(kc == 0),
                    stop=(kc == 3),
                )
                if kc == 3:
                    nc.vector.tensor_scalar_add(
                        pre1[:, t0 : t0 + CH, jc, :], ps[:, 0:CH, :],
                        b1[:, jc : jc + 1],
                    )

            def g0_group(k, jc):
                for kc in range(2):
                    g0_mm(k, jc, kc)

            # Single-MM jobs with even release spreading so ~one fill MM is
            # available every iteration (greedy draining bunches the GEMM work
            # into the early iterations and leaves bare stalls later).
            # (release, deadline, k, jc, kc)
            q0 = deque(
                (CH * (k - 1) + (jc * 2 + kc) * 4, CH * k, k, jc, kc)
                for k in range(1, NCH) for jc in range(4) for kc in range(2)
            )
            q1 = deque(
                (CH * k + CH + ((jc * 4 + kc) * 13) // 16, CH * k + D, k, jc, kc)
                for k in range(NCH) for jc in range(4) for kc in range(4)
            )
            # gps[jc] is shared by q0 and q1: only one open accumulation group
            # per bank (start=True zeroes the whole 2KB region). open0/open1
            # track remaining MMs of an in-flight group per bank.
            open0 = [0] * 4
            open1 = [0] * 4

            def pop(t, n=1):
                for _ in range(n):
                    c0 = q0[0] if q0 and q0[0][0] <= t else None
                    c1 = q1[0] if q1 and q1[0][0] <= t else None
                    if c0 is not None and c0[4] == 0 and open1[c0[3]] > 0:
                        c0 = None  # bank busy with a q1 group
                    if c1 is not None and c1[4] == 0 and open0[c1[3]] > 0:
                        c1 = None
                    if c0 is None and c1 is None:
                        return
                    if c1 is None or (c0 is not None and c0[1] <= c1[1]):
                        _, dl, k, jc, kc = q0.popleft()
                        assert dl > t, f"pre0 late: k={k} jc={jc} t={t}"
                        open0[jc] = 1 if kc == 0 else open0[jc] - 1
                        g0_mm(k, jc, kc)
                    else:
                        _, dl, k, jc, kc = q1.popleft()
                        assert dl > t, f"pre1 late: k={k} jc={jc} t={t}"
                        open1[jc] = 3 if kc == 0 else open1[jc] - 1
                        g1_mm(k, jc, kc)

            def scan_step(t, pre, whh, ps, h_out, h_in_fn):
                sl = t % 4
                if sl == 0:
                    # inject pre for this step AND the next 3 (same PSUM bank)
                    nc.tensor.matmul(
                        ps[:, 0:4, :, :], ident[:], pre[:, t : t + 4, :, :],
                        start=True, stop=False,
                    )
                for kc in range(4):
                    for jc in range(4):
                        nc.tensor.matmul(
                            ps[:, sl, jc, :],
                            whh[:, kc, jc, :],
                            h_in_fn(kc),
                            start=False,
                            stop=(kc == 3),
                        )
                nc.scalar.activation(h_out, ps[:, sl, :, :], Tanh)

            def l0_step(t):
                ps = sps[(t // 4) % 2]
                if t == 0:
                    nc.tensor.matmul(
                        ps[:, 0:4, :, :], ident[:], pre0[:, 0:4, :, :],
                        start=True, stop=False,
                    )
                    nc.scalar.activation(out0[:, 0, :, :], ps[:, 0, :, :], Tanh)
                else:
                    scan_step(
                        t, pre0, whh0, ps,
                        out0[:, t, :, :],
                        lambda kc: out0[:, t - 1, kc, :],
                    )

            def l1_step(t):
                ps = sps[2 + (t // 4) % 2]
                if t == 0:
                    nc.tensor.matmul(
                        ps[:, 0:4, :, :], ident[:], pre1[:, 0:4, :, :],
                        start=True, stop=False,
                    )
                    nc.scalar.activation(h1[:, 0, :, :], ps[:, 0, :, :], Tanh)
                else:
                    scan_step(
                        t, pre1, whh1, ps,
                        h1[:, t % 2, :, :],
                        lambda kc: h1[:, (t - 1) % 2, kc, :],
                    )

            for jc in range(4):
                g0_group(0, jc)

            for t in range(S + D):
                if t < S:
                    l0_step(t)
                pop(t, 1)
                if t >= D:
                    l1_step(t - D)
                pop(t, 1)

            assert not q0 and not q1, (len(q0), len(q1))

            fps = gps[0]
            for kc in range(4):
                nc.tensor.matmul(
                    fps[0:C, 0, :], wfc[:, kc, :], h1[:, 1, kc, :],
                    start=(kc == 0), stop=(kc == 3),
                )
            nc.vector.tensor_scalar_add(fco[:], fps[0:C, 0, :], bfc[:])
            nc.sync.dma_start(out_d[:], fco[:])

    nc.compile()
    return nc


def _prep_inputs(inputs):
    bf = ml_dtypes.bfloat16
    w_ih0 = inputs["w_ih0"]
    w_hh0 = inputs["w_hh0"]
    w_ih1 = inputs["w_ih1"]
    w_hh1 = inputs["w_hh1"]
    w_fc = inputs["w_fc"]

    def lhsT_4(w, n_kc):
        # w: [512, n_kc*128] -> [kp, kc, jc, jp]
        return np.ascontiguousarray(
            w.reshape(4, 128, n_kc, 128).transpose(3, 2, 0, 1)
        ).astype(bf)

    shared = {
        "wih0": lhsT_4(w_ih0, 2),
        "whh0": lhsT_4(w_hh0, 4),
        "wih1": lhsT_4(w_ih1, 4),
        "whh1": lhsT_4(w_hh1, 4),
        "wfc": np.ascontiguousarray(w_fc.reshape(C, 4, 128).transpose(2, 1, 0)).astype(bf),
        "b0": np.ascontiguousarray(
            (inputs["b_ih0"] + inputs["b_hh0"]).reshape(4, 128).T
        ).astype(np.float32),
        "b1": np.ascontiguousarray(
            (inputs["b_ih1"] + inputs["b_hh1"]).reshape(4, 128).T
        ).astype(np.float32),
        "bfc": inputs["b_fc"].reshape(C, 1).astype(np.float32),
        "ident": np.eye(128, dtype=np.float32).astype(bf),
    }
    x = inputs["x"]
    in_maps = []
    for c in range(NCORES):
        xs = x[c * BL : (c + 1) * BL]  # [b, t, i]
        xT = (
            np.ascontiguousarray(
                xs.transpose(2, 1, 0).reshape(2, 128, S * BL).transpose(1, 0, 2)
            )
        ).astype(bf)
        m = dict(shared)
        m["xT"] = xT
        in_maps.append(m)
    return in_maps


def kernel(**inputs):
    from concourse import bass_utils

    if "nc" not in _cache:
        _cache["nc"] = _build_nc()
    nc = _cache["nc"]
    in_maps = _prep_inputs(inputs)
    res = bass_utils.run_bass_kernel_spmd(nc, in_maps, core_ids=list(range(NCORES)))
    y = np.concatenate(
        [np.asarray(res.results[c]["out"]).T for c in range(NCORES)], axis=0
    )
    return y.astype(np.float32)



# revision 17
# speedup vs baseline: 1.0254x; 1.0213x over previous
import numpy as np
import ml_dtypes

B, S, I, H, C = 64, 512, 256, 512, 10
NCORES = 8
BL = B // NCORES
CH = 32
D = 36
NCH = S // CH

_cache = {}


def _build_nc():
    from collections import deque

    import concourse.bass as bass
    import concourse.bacc as bacc
    import concourse.tile as tile
    from concourse.bass import mybir

    f32 = mybir.dt.float32
    bf16 = mybir.dt.bfloat16
    Tanh = mybir.ActivationFunctionType.Tanh

    nc = bacc.Bacc("TRN2", target_bir_lowering=False, debug=False, num_devices=NCORES)

    xT_d = nc.dram_tensor("xT", [128, 2, S * BL], bf16, kind="ExternalInput")
    wih0_d = nc.dram_tensor("wih0", [128, 2, 4, 128], bf16, kind="ExternalInput")
    whh0_d = nc.dram_tensor("whh0", [128, 4, 4, 128], bf16, kind="ExternalInput")
    wih1_d = nc.dram_tensor("wih1", [128, 4, 4, 128], bf16, kind="ExternalInput")
    whh1_d = nc.dram_tensor("whh1", [128, 4, 4, 128], bf16, kind="ExternalInput")
    wfc_d = nc.dram_tensor("wfc", [128, 4, C], bf16, kind="ExternalInput")
    b0_d = nc.dram_tensor("b0", [128, 4], f32, kind="ExternalInput")
    b1_d = nc.dram_tensor("b1", [128, 4], f32, kind="ExternalInput")
    bfc_d = nc.dram_tensor("bfc", [C, 1], f32, kind="ExternalInput")
    id_d = nc.dram_tensor("ident", [128, 128], bf16, kind="ExternalInput")
    out_d = nc.dram_tensor("out", [C, BL], f32, kind="ExternalOutput")

    with tile.TileContext(nc) as tc:
        with tc.tile_pool(name="sb", bufs=1) as sb, tc.tile_pool(
            name="ps", bufs=1, space="PSUM"
        ) as psp:
            xT = sb.tile([128, 2, S * BL], bf16)
            pre0 = sb.tile([128, S, 4, BL], bf16)
            out0 = sb.tile([128, S, 4, BL], bf16)
            pre1 = sb.tile([128, S, 4, BL], bf16)
            wih0 = sb.tile([128, 2, 4, 128], bf16)
            whh0 = sb.tile([128, 4, 4, 128], bf16)
            wih1 = sb.tile([128, 4, 4, 128], bf16)
            whh1 = sb.tile([128, 4, 4, 128], bf16)
            wfc = sb.tile([128, 4, C], bf16)
            b0 = sb.tile([128, 4], f32)
            b1 = sb.tile([128, 4], f32)
            bfc = sb.tile([C, 1], f32)
            ident = sb.tile([128, 128], bf16)
            h1 = sb.tile([128, 2, 4, BL], bf16)
            fco = sb.tile([C, BL], f32)

            nc.sync.dma_start(wih0[:], wih0_d[:])
            nc.sync.dma_start(
                xT[:, :, 0 : CH * BL], xT_d[:, :, 0 : CH * BL]
            )
            nc.sync.dma_start(b0[:], b0_d[:])
            nc.sync.dma_start(ident[:], id_d[:])
            nc.sync.dma_start(
                xT[:, :, CH * BL : 4 * CH * BL], xT_d[:, :, CH * BL : 4 * CH * BL]
            )
            nc.sync.dma_start(whh0[:], whh0_d[:])
            nc.sync.dma_start(
                xT[:, :, 4 * CH * BL :], xT_d[:, :, 4 * CH * BL :]
            )
            for t_sb, t_d in [
                (wih1, wih1_d), (whh1, whh1_d), (b1, b1_d),
                (wfc, wfc_d), (bfc, bfc_d),
            ]:
                nc.sync.dma_start(t_sb[:], t_d[:])

            gps = [psp.tile([128, 64, BL], f32, name=f"gps{i}") for i in range(4)]
            sps = [psp.tile([128, 4, 4, BL], f32, name=f"sps{i}") for i in range(4)]

            def g0_group(k, jc):
                t0 = k * CH
                ps = gps[jc]
                for kc in range(2):
                    nc.tensor.matmul(
                        ps[:, 0:CH, :],
                        wih0[:, kc, jc, :],
                        xT[:, kc, t0 * BL : (t0 + CH) * BL],
                        start=(kc == 0),
                        stop=(kc == 1),
                    )
                nc.vector.tensor_scalar_add(
                    pre0[:, t0 : t0 + CH, jc, :], ps[:, 0:CH, :], b0[:, jc : jc + 1]
                )

            def g1_group(k, jc):
                t0 = k * CH
                ps = gps[jc]
                for kc in range(4):
                    nc.tensor.matmul(
                        ps[:, 0:CH, :],
                        wih1[:, kc, jc, :],
                        out0[:, t0 : t0 + CH, kc, :],
                        start=(kc == 0),
                        stop=(kc == 3),
                    )
                nc.vector.tensor_scalar_add(
                    pre1[:, t0 : t0 + CH, jc, :], ps[:, 0:CH, :], b1[:, jc : jc + 1]
                )

            # (chunk, jc, emit_fn); q1 items gated by min_t
            q0 = deque(
                (k, jc, g0_group) for k in range(1, NCH) for jc in range(4)
            )
            q1 = deque(
                ((k + 1) * CH + 2, k, jc, g1_group) for k in range(NCH) for jc in range(4)
            )

            def pop(t, n=1):
                for _ in range(n):
                    if q1 and q1[0][0] <= t:
                        _, k, jc, fn = q1.popleft()
                        fn(k, jc)
                    elif q0:
                        k, jc, fn = q0.popleft()
                        fn(k, jc)

            def drain_q0(k):
                while q0 and q0[0][0] <= k:
                    kk, jc, fn = q0.popleft()
                    fn(kk, jc)

            def drain_q1(k):
                while q1 and q1[0][1] <= k:
                    _, kk, jc, fn = q1.popleft()
                    fn(kk, jc)

            def scan_step(t, pre, whh, ps, h_out, h_in_fn):
                sl = t % 4
                if sl == 0:
                    # inject pre for this step AND the next 3 (same PSUM bank)
                    nc.tensor.matmul(
                        ps[:, 0:4, :, :], ident[:], pre[:, t : t + 4, :, :],
                        start=True, stop=False,
                    )
                for kc in range(4):
                    for jc in range(4):
                        nc.tensor.matmul(
                            ps[:, sl, jc, :],
                            whh[:, kc, jc, :],
                            h_in_fn(kc),
                            start=False,
                            stop=(kc == 3),
                        )
                nc.scalar.activation(h_out, ps[:, sl, :, :], Tanh)

            def l0_step(t):
                ps = sps[(t // 4) % 2]
                if t == 0:
                    nc.tensor.matmul(
                        ps[:, 0:4, :, :], ident[:], pre0[:, 0:4, :, :],
                        start=True, stop=False,
                    )
                    nc.scalar.activation(out0[:, 0, :, :], ps[:, 0, :, :], Tanh)
                else:
                    scan_step(
                        t, pre0, whh0, ps,
                        out0[:, t, :, :],
                        lambda kc: out0[:, t - 1, kc, :],
                    )

            def l1_step(t):
                ps = sps[2 + (t // 4) % 2]
                if t == 0:
                    nc.tensor.matmul(
                        ps[:, 0:4, :, :], ident[:], pre1[:, 0:4, :, :],
                        start=True, stop=False,
                    )
                    nc.scalar.activation(h1[:, 0, :, :], ps[:, 0, :, :], Tanh)
                else:
                    scan_step(
                        t, pre1, whh1, ps,
                        h1[:, t % 2, :, :],
                        lambda kc: h1[:, (t - 1) % 2, kc, :],
                    )

            for jc in range(4):
                g0_group(0, jc)

            for t in range(S + D):
                if t < S:
                    drain_q0(t // CH)
                    l0_step(t)
                pop(t, 1)
                if t >= D:
                    s = t - D
                    drain_q1(s // CH)
                    l1_step(s)
                    pop(t, 1)

            fps = gps[0]
            for kc in range(4):
                nc.tensor.matmul(
                    fps[0:C, 0, :], wfc[:, kc, :], h1[:, 1, kc, :],
                    start=(kc == 0), stop=(kc == 3),
                )
            nc.vector.tensor_scalar_add(fco[:], fps[0:C, 0, :], bfc[:])
            nc.sync.dma_start(out_d[:], fco[:])

    nc.compile()
    return nc


def _prep_inputs(inputs):
    bf = ml_dtypes.bfloat16
    w_ih0 = inputs["w_ih0"]
    w_hh0 = inputs["w_hh0"]
    w_ih1 = inputs["w_ih1"]
    w_hh1 = inputs["w_hh1"]
    w_fc = inputs["w_fc"]

    def lhsT_4(w, n_kc):
        # w: [512, n_kc*128] -> [kp, kc, jc, jp]
        return np.ascontiguousarray(
            w.reshape(4, 128, n_kc, 128).transpose(3, 2, 0, 1)
        ).astype(bf)

    shared = {
        "wih0": lhsT_4(w_ih0, 2),
        "whh0": lhsT_4(w_hh0, 4),
        "wih1": lhsT_4(w_ih1, 4),
        "whh1": lhsT_4(w_hh1, 4),
        "wfc": np.ascontiguousarray(w_fc.reshape(C, 4, 128).transpose(2, 1, 0)).astype(bf),
        "b0": np.ascontiguousarray(
            (inputs["b_ih0"] + inputs["b_hh0"]).reshape(4, 128).T
        ).astype(np.float32),
        "b1": np.ascontiguousarray(
            (inputs["b_ih1"] + inputs["b_hh1"]).reshape(4, 128).T
        ).astype(np.float32),
        "bfc": inputs["b_fc"].reshape(C, 1).astype(np.float32),
        "ident": np.eye(128, dtype=np.float32).astype(bf),
    }
    x = inputs["x"]
    in_maps = []
    for c in range(NCORES):
        xs = x[c * BL : (c + 1) * BL]  # [b, t, i]
        xT = (
            np.ascontiguousarray(
                xs.transpose(2, 1, 0).reshape(2, 128, S * BL).transpose(1, 0, 2)
            )
        ).astype(bf)
        m = dict(shared)
        m["xT"] = xT
        in_maps.append(m)
    return in_maps


def kernel(**inputs):
    from concourse import bass_utils

    if "nc" not in _cache:
        _cache["nc"] = _build_nc()
    nc = _cache["nc"]
    in_maps = _prep_inputs(inputs)
    res = bass_utils.run_bass_kernel_spmd(nc, in_maps, core_ids=list(range(NCORES)))
    y = np.concatenate(
        [np.asarray(res.results[c]["out"]).T for c in range(NCORES)], axis=0
    )
    return y.astype(np.float32)

